# revision 44
# baseline (speedup 1.0000x reference)
"""Trainium2 Bass kernel for CausalSelfAttention (B=2, T=4096, C=1024, 16 heads, RoPE).

Sharding: tensor-parallel across heads. Core c handles heads {2c, 2c+1} for
both batches; the two batches are processed as two "units".

Per core (v2 — fully pipelined, emission interleaved):
  - QKV (emitted as small "pieces" drained into the attention stream so the
    PE never idles): qT/kT computed transposed ([dims, T]) from the PE
    (lhsT = W tiles, rhs = chunk-major x tiles, one 128-descriptor DMA per
    chunk), v computed natural ([T, dims]).  RoPE: qa = pA + b (DVE, bf16),
    partition-rotated copy via 4 SBUF->SBUF DMAs on the gpsimd queue (sign
    folded into the bf16 sin table), two DVE multiplies and an add.
  - Attention paces ScalarE: per 128-k-position chunk, one row-tiled score
    matmul pair (both heads on PE tiles T0/T8) into a [128, 2, 512] PSUM
    strip (double-buffered), exp on ScalarE with the free-dim range narrowed
    on diagonal chunks, triangle mask on the leading 128 columns of diagonal
    chunks only, then K=128 AV matmuls (one per head, deferred one chunk to
    decouple the PSUM-accumulator recycle from the PE FIFO) accumulating y
    and the softmax denominator (65th ones column on v) into a single
    [128, 2, 512] PSUM accumulator.
  - Epilogue: fast evacuation of the accumulator (bf16 y copy + f32 denom
    row), reciprocal_approx_fast, gpsimd partition-broadcast, two aligned
    DVE multiplies into per-head yT halves.
  - Four 8-way AllToAlls (one per unit-half, y^T head-sharded ->
    token-sharded) so only the last 0.5 MB collective is exposed; all proj
    m-tiles (y @ Wp + bp) are deferred to fill that collective's latency.

PSUM budget (8 banks): 2x2 score strips + 2 AV accumulator + 2 qkv/proj ring.
kernel() takes the full unsharded inputs and returns the full output.
"""

import numpy as np
import ml_dtypes

import concourse.bass as bass
import concourse.bacc as bacc
import concourse.mybir as mybir
import concourse.tile as tile

BF16 = mybir.dt.bfloat16
F32 = mybir.dt.float32
NPBF16 = ml_dtypes.bfloat16

N_EMBD = 1024
N_HEAD = 16
HS = 64
B = 2
T_FULL = 4096
QT = 512            # q-tile width
KTILE = 128         # k positions per chunk
N_CORES = 8

AluAdd = mybir.AluOpType.add
AluMult = mybir.AluOpType.mult


def build_nc(T=T_FULL, debug=False):
    assert T % QT == 0
    NQT = T // QT          # q-tiles per unit (= per batch)
    NT = T // KTILE        # 128-wide k tiles per unit
    nc = bacc.Bacc()
    if debug:
        yT_dbg = nc.declare_dram_parameter("yT_dbg", [B, 128, T], BF16, isOutput=True)
        qk_dbg = nc.declare_dram_parameter("qk_dbg", [2, 128, T], BF16, isOutput=True)
        rc_dbg = nc.declare_dram_parameter("rc_dbg", [2, 2, QT], F32, isOutput=True)

    xT_d = nc.declare_dram_parameter("xT", [B * T // QT, 128, 8, QT], BF16, isOutput=False)
    wq_d = nc.declare_dram_parameter("Wq", [128, 8, 128], BF16, isOutput=False)
    wk_d = nc.declare_dram_parameter("Wk", [128, 8, 128], BF16, isOutput=False)
    wv_d = nc.declare_dram_parameter("Wv", [128, 8, 128], BF16, isOutput=False)
    bq_d = nc.declare_dram_parameter("bq", [128, 2], F32, isOutput=False)
    bk_d = nc.declare_dram_parameter("bk", [128, 2], F32, isOutput=False)
    bv_d = nc.declare_dram_parameter("bv_bc", [128, 128], F32, isOutput=False)
    cos_d = nc.declare_dram_parameter("cosT", [128, T], BF16, isOutput=False)
    sin_d = nc.declare_dram_parameter("sinT", [128, T], BF16, isOutput=False)
    tri_d = nc.declare_dram_parameter("tri", [128, 2, 128], BF16, isOutput=False)
    wp_d = nc.declare_dram_parameter("Wp", [128, 8, N_EMBD], BF16, isOutput=False)
    bp_d = nc.declare_dram_parameter("bp_bc", [128, N_EMBD], BF16, isOutput=False)
    # out rows: [unit, 512 tokens of this core's q-block, N_EMBD]
    out_d = nc.declare_dram_parameter("out", [B, QT, N_EMBD], F32, isOutput=True)

    rec_d = nc.dram_tensor("rec_scratch", [B, 8, 2 * QT], F32)
    sync_in = nc.dram_tensor("sync_in", [8, 1, 16], BF16)
    sync_out = nc.dram_tensor("sync_out", [8, 1, 16], BF16)
    a2a_in = [[nc.dram_tensor(f"a2a_in{u}{hf}", [8, 128, 256], BF16) for hf in range(2)]
              for u in range(B)]
    a2a_out = [[nc.dram_tensor(f"a2a_out{u}{hf}", [8, 128, 256], BF16) for hf in range(2)]
               for u in range(B)]

    with tile.TileContext(nc) as tc, \
         tc.tile_pool(name="const", bufs=1) as const, \
         tc.tile_pool(name="persist", bufs=1) as persist, \
         tc.tile_pool(name="xc", bufs=4) as xpool, \
         tc.tile_pool(name="stage", bufs=3) as stage, \
         tc.tile_pool(name="ppool", bufs=3) as ppool, \
         tc.tile_pool(name="epi", bufs=2) as epi, \
         tc.tile_pool(name="projp", bufs=2) as projp, \
         tc.tile_pool(name="qkv_ps", bufs=2, space="PSUM") as qkv_ps, \
         tc.tile_pool(name="strip_ps", bufs=2, space="PSUM") as strip_ps, \
         tc.tile_pool(name="av_ps", bufs=1, space="PSUM") as av_ps:

        # ---- constants (spread across queues; xc chunk DMAs go on sync) ----
        wq_sb = const.tile([128, 8, 128], BF16, tag="wq")
        wk_sb = const.tile([128, 8, 128], BF16, tag="wk")
        wv_sb = const.tile([128, 8, 128], BF16, tag="wv")
        nc.scalar.dma_start(out=wq_sb[:], in_=wq_d[:])
        cos_sb = const.tile([128, T], BF16, tag="cos")
        sin_sb = const.tile([128, T], BF16, tag="sin")
        bq_sb = const.tile([128, 2], F32, tag="bq")
        nc.scalar.dma_start(out=bq_sb[:], in_=bq_d[:])
        nc.scalar.dma_start(out=cos_sb[:], in_=cos_d[:])
        nc.scalar.dma_start(out=sin_sb[:], in_=sin_d[:])
        nc.scalar.dma_start(out=wk_sb[:], in_=wk_d[:])
        nc.scalar.dma_start(out=wv_sb[:], in_=wv_d[:])
        bk_sb = const.tile([128, 2], F32, tag="bk")
        bv_sb = const.tile([128, 128], F32, tag="bv")
        nc.scalar.dma_start(out=bk_sb[:], in_=bk_d[:])
        nc.scalar.dma_start(out=bv_sb[:], in_=bv_d[:])
        tri_sb = const.tile([128, 2, 128], BF16, tag="tri")
        nc.scalar.dma_start(out=tri_sb[:], in_=tri_d[:])
        wp_sb = const.tile([128, 8, N_EMBD], BF16, tag="wp")
        nc.gpsimd.dma_start(out=wp_sb[:], in_=wp_d[:])
        bp_sb = const.tile([128, N_EMBD], BF16, tag="bp")
        nc.gpsimd.dma_start(out=bp_sb[:], in_=bp_d[:])

        # ---- persistent per-unit tensors ----
        qT = [persist.tile([128, T], BF16, tag=f"qT{u}", name=f"qT{u}") for u in range(B)]
        kT = [persist.tile([128, T], BF16, tag=f"kT{u}", name=f"kT{u}") for u in range(B)]
        vP = [persist.tile([128, NT, 130], BF16, tag=f"vP{u}", name=f"vP{u}") for u in range(B)]
        # y^T split per head so the normalize multiplies stay partition-aligned
        yTh = [[persist.tile([64, T], BF16, tag=f"yT{u}{h}", name=f"yT{u}{h}")
                for h in range(2)] for u in range(B)]
        for u in range(B):
            nc.vector.memset(vP[u][:, :, 64:65], 1.0)
            nc.vector.memset(vP[u][:, :, 129:130], 1.0)
        # tiny warmup all-to-all: absorbs boot-time core skew during the idle
        # head so the first real collective doesn't stall mid-attention
        nc.gpsimd.collective_compute(
            "AllToAll", mybir.AluOpType.bypass,
            replica_groups=[[0, 1, 2, 3, 4, 5, 6, 7]],
            ins=[sync_in[:]], outs=[sync_out[:]],
        )

        def qkv_chunk_pieces(u, ch):
            """Pieces (closures) computing qT/kT/vP for tokens [ch*512,(ch+1)*512) of unit u."""
            st = {}

            def p_xc():
                chg = u * (T // QT) + ch
                xc = xpool.tile([128, 8, QT], BF16, tag="xc", name="xc")
                nc.sync.dma_start(out=xc[:], in_=xT_d[chg])
                st["xc"] = xc

            def mk_qk(w_sb, b_sb, dstT):
                def p_qk():
                    csl = slice(ch * QT, (ch + 1) * QT)
                    xc = st["xc"]
                    pA = qkv_ps.tile([128, QT], F32, tag="qk", name="pA")
                    for ct in range(8):
                        nc.tensor.matmul(pA[:], w_sb[:, ct, :], xc[:, ct, :],
                                         start=(ct == 0), stop=(ct == 7))
                    qa = stage.tile([128, QT], BF16, tag="qa", name="qa", bufs=3)
                    nc.vector.tensor_scalar_add(qa[:], pA[:], b_sb[:, 0:1])
                    qr = stage.tile([128, QT], BF16, tag="qr", name="qr", bufs=3)
                    for (dp, sp) in ((0, 32), (32, 0), (64, 96), (96, 64)):
                        nc.sync.dma_start(out=qr[dp:dp + 32, :], in_=qa[sp:sp + 32, :])
                    m1 = stage.tile([128, QT], BF16, tag="m1", name="m1", bufs=2)
                    m2 = stage.tile([128, QT], BF16, tag="m2", name="m2", bufs=2)
                    nc.vector.tensor_mul(m1[:], qa[:], cos_sb[:, csl])
                    nc.vector.tensor_mul(m2[:], qr[:], sin_sb[:, csl])
                    nc.vector.tensor_add(dstT[:, csl], m1[:], m2[:])
                return p_qk

            def mk_v(t4):
                def p_v():
                    xc = st["xc"]
                    ttg = ch * 4 + t4
                    tsl = slice(t4 * 128, (t4 + 1) * 128)
                    pV = qkv_ps.tile([128, QT], F32, tag="qk", name="pV")
                    for ct in range(8):
                        nc.tensor.matmul(pV[:, 0:128], xc[:, ct, tsl], wv_sb[:, ct, :],
                                         start=(ct == 0), stop=(ct == 7))
                    pv2 = bass.AP(tensor=pV.tensor, offset=pV.offset,
                                  ap=[list(pV.ap[0]), [64, 2], [1, 64]])
                    dst = vP[u][:, ttg, 0:129]
                    dst2 = bass.AP(tensor=dst.tensor, offset=dst.offset,
                                   ap=[list(dst.ap[0]), [65, 2], [1, 64]])
                    bv2 = bass.AP(tensor=bv_sb.tensor, offset=bv_sb.offset,
                                  ap=[list(bv_sb.ap[0]), [64, 2], [1, 64]])
                    nc.vector.tensor_add(dst2, pv2, bv2)
                return p_v

            return [p_xc, mk_qk(wq_sb, bq_sb, qT[u]), mk_qk(wk_sb, bk_sb, kT[u])] + \
                [mk_v(t4) for t4 in range(4)]

        def attn_block(u, j, drain):
            """Attention for q-tile j (512 q) of unit u against k tiles 0..4j+3.
            Calls drain() between chunks to interleave background PE work."""
            jsl = slice(j * QT, (j + 1) * QT)
            nchunks = 4 * (j + 1)
            av_t = av_ps.tile([128, 2, QT], F32, tag="av", name="av_t")

            def av_emit(pend):
                P, c, qoff, w = pend
                first, last = (c == 0), (c == nchunks - 1)
                for h in range(2):
                    nc.tensor.matmul(av_t[0:65, h, qoff:QT],
                                     vP[u][:, c, 65 * h:65 * h + 65],
                                     P[:, h, 0:w],
                                     start=first, stop=last)

            pend = None
            for c in range(nchunks):
                s = c - (nchunks - 4)          # diagonal sub-position 0..3, or <0
                qoff = 128 * s if s > 0 else 0
                w = QT - qoff
                ksl = slice(c * KTILE, (c + 1) * KTILE)
                strip = strip_ps.tile([128, 2, QT], F32, tag="strip", name="strip")
                for h in range(2):
                    hsl = slice(64 * h, 64 * (h + 1))
                    nc.tensor.matmul(strip[:, h, 0:w], kT[u][hsl, ksl],
                                     qT[u][hsl, j * QT + qoff:(j + 1) * QT],
                                     start=True, stop=True)
                P = ppool.tile([128, 2, QT], BF16, tag="P", name="P", bufs=4)
                nc.scalar.activation(P[:, :, 0:w], strip[:, :, 0:w],
                                     mybir.ActivationFunctionType.Exp)
                if s >= 0:  # leading 128 cols of a diagonal chunk: triangle mask
                    nc.vector.tensor_mul(P[:, :, 0:128], P[:, :, 0:128], tri_sb[:])
                if pend is not None:
                    av_emit(pend)
                pend = (P, c, qoff, w)
                if c % 3 == 2:
                    drain(1)
            av_emit(pend)
            # epilogue: evacuate av_t fast (frees the PSUM accumulator for the
            # next block), then normalize off the critical path.
            yc = epi.tile([64, 2, QT], BF16, tag="yc", name="yc")
            nc.vector.tensor_copy(yc[:], av_t[0:64, :, :])
            den = epi.tile([1, 2, QT], F32, tag="den", name="den")
            nc.vector.tensor_copy(den[:], av_t[64:65, :, :])
            rc = epi.tile([1, 2, QT], F32, tag="rc", name="rc")
            rb = epi.tile([64, 2, QT], F32, tag="rb", name="rb", bufs=1)
            nc.vector.reciprocal_approx_fast(rc[0:1, 0, :], den[0:1, 0, :])
            nc.vector.reciprocal_approx_fast(rc[0:1, 1, :], den[0:1, 1, :])
            nc.sync.dma_start(out=rec_d[u, j], in_=rc[0:1, :, :])
            dsrc = rec_d[u, j]
            bsrc = bass.AP(tensor=dsrc.tensor, offset=dsrc.offset,
                           ap=[[0, 64]] + list(dsrc.ap))
            nc.sync.dma_start(out=rb[:], in_=bsrc)
            for h in range(2):
                nc.vector.tensor_mul(yTh[u][h][:, jsl], yc[:, h, :], rb[:, h, :])
                hf = j // 4
                for dh in range(2):
                    nc.sync.dma_start(
                        out=a2a_in[u][hf][2 * (j % 4) + dh, 64 * h:64 * (h + 1), :],
                        in_=yTh[u][h][:, j * QT + 256 * dh:j * QT + 256 * (dh + 1)])
            if debug and j == 1:
                nc.sync.dma_start(out=rc_dbg[u], in_=rc[0:1, :, :])

        def a2a_start(u, hf):
            nc.gpsimd.collective_compute(
                "AllToAll", mybir.AluOpType.bypass,
                replica_groups=[[0, 1, 2, 3, 4, 5, 6, 7]],
                ins=[a2a_in[u][hf][:]], outs=[a2a_out[u][hf][:]],
            )

        def proj_mtile(u, hf, m):
            """out rows for tokens [m*128,(m+1)*128) of this core's 256-token
            block of half hf of unit u."""
            msl = slice(m * 128, (m + 1) * 128)
            ydm = projp.tile([128, 8, 128], BF16, tag="ydm", name="ydm")
            # dst[d, s, t] = a2a_out[u][hf][s, d, m*128 + t]  (one DMA)
            src = a2a_out[u][hf][0, 0:128, msl]
            src3 = bass.AP(tensor=src.tensor, offset=src.offset,
                           ap=[list(src.ap[0]), [128 * 256, 8], list(src.ap[1])])
            nc.scalar.dma_start(out=ydm[:], in_=src3)
            ob = projp.tile([128, N_EMBD], F32, tag="ob", name="ob", bufs=1)
            for nh in range(2):
                nsl = slice(nh * 512, (nh + 1) * 512)
                pp = qkv_ps.tile([128, QT], F32, tag="qk", name="pp")
                for ft in range(8):
                    nc.tensor.matmul(pp[:], ydm[:, ft, :], wp_sb[:, ft, nsl],
                                     start=(ft == 0), stop=(ft == 7))
                nc.vector.tensor_add(ob[:, nsl], pp[:], bp_sb[:, nsl])
            nc.sync.dma_start(out=out_d[u, hf * 256 + m * 128:hf * 256 + (m + 1) * 128, :],
                              in_=ob[:])

        def proj_pieces(u, hf):
            return [(lambda u=u, hf=hf, m=m: proj_mtile(u, hf, m)) for m in range(2)]

        # ---- schedule: attention paces ScalarE; qkv/proj pieces fill PE gaps ----
        bg = []          # list of (key, piece_fn); key=(u, ch) for qkv, None otherwise
        bgi = [0]
        qkv_done = {}    # u -> highest chunk fully emitted

        def drain(n):
            for _ in range(n):
                if bgi[0] >= len(bg):
                    return
                key, fn = bg[bgi[0]]
                bgi[0] += 1
                fn()
                if key is not None:
                    qkv_done[key[0]] = key[1]

        def add_chunk(u, ch):
            ps = qkv_chunk_pieces(u, ch)
            # key only on the LAST piece: chunk counts as emitted when all pieces ran
            bg.extend((None, p) for p in ps[:-1])
            bg.append(((u, ch), ps[-1]))

        ps0 = qkv_chunk_pieces(0, 0)
        for p in ps0:
            p()
        qkv_done[0] = 0
        for ch in range(1, NQT):
            add_chunk(0, ch)
        for ch in range(NQT):
            add_chunk(1, ch)

        for u in range(B):
            for j in range(NQT):
                while qkv_done.get(u, -1) < j:
                    drain(1)
                attn_block(u, j, drain)
                if j == 3:
                    a2a_start(u, 0)
            a2a_start(u, 1)
        while bgi[0] < len(bg):
            drain(1)
        # earlier halves' proj fills the last collective's latency window
        for (u, hf) in ((0, 0), (0, 1), (1, 0), (1, 1)):
            for p in proj_pieces(u, hf):
                p()
        if debug:
            for u in range(B):
                for h in range(2):
                    nc.sync.dma_start(out=yT_dbg[u, 64 * h:64 * (h + 1), :],
                                      in_=yTh[u][h][:])
            nc.sync.dma_start(out=qk_dbg[0], in_=qT[0][:])
            nc.sync.dma_start(out=qk_dbg[1], in_=kT[0][:])

    nc.compile()
    return nc


def make_inputs(x, W_attn, b_attn, W_proj, b_proj, T):
    """Build the 8 per-core input maps from full inputs."""
    scale = 1.0 / np.sqrt(HS)
    inv_freq = 1.0 / (10000.0 ** (np.arange(0, HS, 2, dtype=np.float64) / HS))  # [32]
    t = np.arange(T, dtype=np.float64)
    freqs = np.outer(t, inv_freq)  # [T, 32]
    rows = np.arange(128)
    cosT = np.cos(freqs[:, rows % 32]).T.astype(np.float32)  # [128, T]
    sinT = np.sin(freqs[:, rows % 32]).T.astype(np.float32)
    sign = np.where((rows % 64) < 32, -1.0, 1.0).astype(np.float32)[:, None]
    sinT = sinT * sign

    # triangle mask for the leading 128 cols of diagonal chunks: 1 iff p <= f
    p = np.arange(128)[:, None]
    f = np.arange(128)[None, :]
    tri = (p <= f).astype(np.float32)
    tri2 = np.stack([tri, tri], axis=1)  # [128, 2, 128]

    C = N_EMBD
    # chunk-major x: xh[ch, p, a, t] = x[ch*QT + t, a*128 + p]
    xh = np.ascontiguousarray(
        x.reshape(B * T // QT, QT, 8, 128).transpose(0, 3, 2, 1)).astype(NPBF16)
    # rot permutation of head dims: d -> d+32 (first half) / d-32 (second half)
    d = np.arange(128)
    perm = np.where((d % 64) < 32, d + 32, d - 32)
    bp_bc = np.broadcast_to(b_proj[None, :], (128, N_EMBD)).astype(np.float32).copy()
    in_maps = []
    for c in range(N_CORES):
        hsl = slice(128 * c, 128 * (c + 1))  # dims of heads {2c, 2c+1}
        Wq = W_attn[:, 0 * C:1 * C][:, hsl] * scale
        Wk = W_attn[:, 1 * C:2 * C][:, hsl]
        Wv = W_attn[:, 2 * C:3 * C][:, hsl]
        bq = (b_attn[0 * C:1 * C][hsl] * scale).astype(np.float32)
        bk = b_attn[1 * C:2 * C][hsl].astype(np.float32)
        bv = b_attn[2 * C:3 * C][hsl]
        in_maps.append({
            "xT": xh,
            "Wq": np.ascontiguousarray(Wq.reshape(8, 128, 128).transpose(1, 0, 2)).astype(NPBF16),
            "Wk": np.ascontiguousarray(Wk.reshape(8, 128, 128).transpose(1, 0, 2)).astype(NPBF16),
            "Wv": np.ascontiguousarray(Wv.reshape(8, 128, 128).transpose(1, 0, 2)).astype(NPBF16),
            "bq": np.stack([bq, bq[perm]], axis=1).copy(),
            "bk": np.stack([bk, bk[perm]], axis=1).copy(),
            "bv_bc": np.broadcast_to(bv[None, :], (128, 128)).astype(np.float32).copy(),
            "cosT": cosT.astype(NPBF16),
            "sinT": sinT.astype(NPBF16),
            "tri": tri2.astype(NPBF16),
            "Wp": np.ascontiguousarray(W_proj.reshape(8, 128, N_EMBD).transpose(1, 0, 2)).astype(NPBF16),
            "bp_bc": bp_bc.astype(NPBF16),
        })
    return in_maps


def assemble(results, T):
    out = np.empty((B, T, N_EMBD), dtype=np.float32)
    for c in range(N_CORES):
        blk = results[c]["out"]  # [B, 512, N_EMBD]: [hf*256 + r] rows
        for u in range(B):
            for hf in range(2):
                out[u, hf * 2048 + c * 256:hf * 2048 + (c + 1) * 256, :] =                     blk[u, hf * 256:(hf + 1) * 256]
    return out


_NC_CACHE = {}


def kernel(x, W_attn, b_attn, W_proj, b_proj):
    from concourse.bass_utils import run_bass_kernel_spmd
    x = np.asarray(x, dtype=np.float32)
    W_attn = np.asarray(W_attn, dtype=np.float32)
    b_attn = np.asarray(b_attn, dtype=np.float32)
    W_proj = np.asarray(W_proj, dtype=np.float32)
    b_proj = np.asarray(b_proj, dtype=np.float32)
    T = x.shape[1]
    if T not in _NC_CACHE:
        _NC_CACHE[T] = build_nc(T)
    nc = _NC_CACHE[T]
    in_maps = make_inputs(x, W_attn, b_attn, W_proj, b_proj, T)
    res = run_bass_kernel_spmd(nc, in_maps, core_ids=list(range(N_CORES)))
    return assemble(res.results, T)
